# revision 15
# baseline (speedup 1.0000x reference)
"""Multi-head attention (b=2, t=2048, h=16, dh=128, d_model=2048) on 8 TRN2 cores.

Sharding: core c -> batch c//4, head group g=c%4 (heads [4g, 4g+4)).  Each core
computes QKV projections for its 4 heads, causal attention, and a partial
output projection (contraction over its heads).  The host sums the 4 partials
per batch and adds bo.

Faithful to the reference's reshape quirk: q = (x@Wq+bq).reshape(b, h, t, dh)
is a raw reshape, so q-head h is rows [128h, 128h+128) of the Q projection
buffer (all 2048 cols), reinterpreted row-major as (t=2048, dh=128).  Hence
q_h^T[d, t] = slab_h[t//16, 128*(t%16)+d]: the slab's 16 column blocks are
PE-transposed into T[d, j, r] and q_h^T tiles are addressed as
T[:, :, r-slice] with free dims permuted to (r, j).

Attention is computed transposed (S^T[s, t]) so the AV matmul needs no
transposes; the softmax denominator comes from a ones-vector matmul and is
applied via a reciprocal broadcast matmul + DVE multiply.  Causal masking is
an affine_select zeroing exp(S^T) where s > t (softmax omits the max
subtraction: logits here are bounded by ~|6| so exp is safe, matching the
reference to fp32 accuracy).

All matmuls run in float32r (full PE rate at moving dim 512, ~1.6e-4 rel
err) with fp32 PSUM accumulation.
"""

import sys

sys.path.insert(0, "/opt/trn_rl_repo")

import numpy as np
from contextlib import ExitStack

import concourse.bass as bass
import concourse.tile as tile
from concourse import bacc, mybir
from concourse.bass import ds
from concourse.bass_utils import run_bass_kernel_spmd
from concourse.masks import make_identity

P = 128
T = 2048
D = 2048           # d_model
H_PER_CORE = 4
DH = 128
NT = 512           # matmul moving free dim
M_CHUNKS = D // P  # 16 contraction chunks
S_TILES = T // P   # 16 token tiles of 128
TT_TILES = T // NT  # 4 query tiles of 512
N_GROUPS = 4       # x^T streamed in groups of 4 chunks
SCALE = float(1.0 / np.sqrt(DH))

F32 = mybir.dt.float32
F32R = mybir.dt.float32r

_CACHE = {}


def _build():
    nc = bacc.Bacc(name="mha8")

    x_t = nc.dram_tensor("x_t", (D, T), F32R, kind="ExternalInput")     # x[b].T
    x_slab = nc.dram_tensor("x_slab", (D, H_PER_CORE * P), F32R,
                            kind="ExternalInput")  # x[b].T cols [512g, 512g+512)
    wq = nc.dram_tensor("wq", (D, D), F32R, kind="ExternalInput")
    wk = nc.dram_tensor("wk", (D, H_PER_CORE * DH), F32R, kind="ExternalInput")
    wv = nc.dram_tensor("wv", (D, H_PER_CORE * DH), F32R, kind="ExternalInput")
    wo = nc.dram_tensor("wo", (H_PER_CORE * DH, D), F32R, kind="ExternalInput")
    bq = nc.dram_tensor("bq", (1, D), F32R, kind="ExternalInput")
    bk = nc.dram_tensor("bk", (1, H_PER_CORE * DH), F32R, kind="ExternalInput")
    bv = nc.dram_tensor("bv", (1, H_PER_CORE * DH), F32R, kind="ExternalInput")
    out = nc.dram_tensor("out", (T, D), F32, kind="ExternalOutput")

    with tile.TileContext(nc) as tc, ExitStack() as top:
        const = top.enter_context(tc.tile_pool(name="const", bufs=1))
        ones_f = const.tile([P, NT], F32, name="ones_f")
        nc.vector.memset(ones_f[:], 1.0)
        ones = const.tile([P, NT], F32R, name="ones")
        nc.vector.tensor_copy(ones[:], ones_f[:])
        ident = const.tile([P, P], F32, name="ident")
        make_identity(nc, ident[:])
        bq_sb = const.tile([1, D], F32R, name="bq_sb")
        nc.sync.dma_start(bq_sb[:], bq[:])
        bk_sb = const.tile([1, H_PER_CORE * DH], F32R, name="bk_sb")
        nc.sync.dma_start(bk_sb[:], bk[:])
        bv_sb = const.tile([1, H_PER_CORE * DH], F32R, name="bv_sb")
        nc.sync.dma_start(bv_sb[:], bv[:])

        acc = top.enter_context(tc.tile_pool(name="acc", bufs=1))
        kacc = [acc.tile([P, T], F32R, name=f"kacc{h}") for h in range(H_PER_CORE)]
        vacc = [acc.tile([P, NT], F32R, name=f"vacc{s}") for s in range(S_TILES)]
        tq_pool = top.enter_context(tc.tile_pool(name="tq", bufs=1))
        tq = [tq_pool.tile([P, M_CHUNKS, P], F32R, name=f"tq{h}")
              for h in range(H_PER_CORE)]

        # ------------------------------------------------------------------
        # Phase 1: projections.  Stream x^T in 4 groups of 4 chunks;
        # psum-accumulate over each group's chunks, then copy/add into the
        # SBUF accumulators.
        # ------------------------------------------------------------------
        with ExitStack() as ph1:
            slab_pool = ph1.enter_context(tc.tile_pool(name="slab", bufs=1))
            slabs = [slab_pool.tile([P, D], F32, name=f"slab{h}")
                     for h in range(H_PER_CORE)]

            with ExitStack() as ph1a:
                xh = ph1a.enter_context(tc.tile_pool(name="xh", bufs=4))
                wst = ph1a.enter_context(tc.tile_pool(name="wst", bufs=3))
                pp = ph1a.enter_context(
                    tc.tile_pool(name="pp", bufs=8, space="PSUM"))

                for g in range(N_GROUPS):
                    first, last = g == 0, g == N_GROUPS - 1
                    xts, xsl = [], []
                    for mi in range(4):
                        m = 4 * g + mi
                        xt = xh.tile([P, T], F32R, tag="xchunk", name=f"x{m}")
                        nc.sync.dma_start(xt[:], x_t[ds(P * m, P), :])
                        xts.append(xt)
                        xs = xh.tile([P, H_PER_CORE * P], F32R, tag="xslab",
                                     name=f"xs{m}")
                        nc.sync.dma_start(xs[:], x_slab[ds(P * m, P), :])
                        xsl.append(xs)

                    # --- K^T: waves of (2 heads x 4 s-tiles of 512) ---
                    for hw in range(2):
                        pts = [[pp.tile([P, NT], F32, tag="pw",
                                        name=f"kps{g}_{hw}_{hh}_{j}")
                                for j in range(4)] for hh in range(2)]
                        for mi in range(4):
                            m = 4 * g + mi
                            wkt = wst.tile([P, 2 * DH], F32R, tag="wk",
                                           name=f"wk{g}_{hw}_{m}")
                            nc.sync.dma_start(
                                wkt[:], wk[ds(P * m, P), ds(2 * DH * hw, 2 * DH)])
                            for hh in range(2):
                                for j in range(4):
                                    nc.tensor.matmul(
                                        pts[hh][j][:],
                                        wkt[:, ds(DH * hh, DH)],
                                        xts[mi][:, ds(NT * j, NT)],
                                        start=(mi == 0),
                                        stop=(not last and mi == 3),
                                    )
                        for hh in range(2):
                            h = 2 * hw + hh
                            for j in range(4):
                                if last:
                                    nc.tensor.matmul(
                                        pts[hh][j][:],
                                        bk_sb[:, ds(DH * h, DH)],
                                        ones[0:1, :],
                                        start=False, stop=True)
                                dst = kacc[h][:, ds(NT * j, NT)]
                                if first:
                                    nc.vector.tensor_copy(dst, pts[hh][j][:])
                                else:
                                    nc.vector.tensor_tensor(
                                        dst, dst, pts[hh][j][:],
                                        mybir.AluOpType.add)

                    # --- V: waves of 8 s-tiles of 128 ---
                    for sw in range(2):
                        pts_v = [pp.tile([P, NT], F32, tag="pw",
                                         name=f"vps{g}_{sw}_{si}")
                                 for si in range(8)]
                        for mi in range(4):
                            m = 4 * g + mi
                            wvt = wst.tile([P, NT], F32R, tag="wv",
                                           name=f"wv{g}_{sw}_{m}")
                            nc.sync.dma_start(wvt[:], wv[ds(P * m, P), :])
                            for si in range(8):
                                s = 8 * sw + si
                                nc.tensor.matmul(
                                    pts_v[si][:],
                                    xts[mi][:, ds(P * s, P)],
                                    wvt[:],
                                    start=(mi == 0),
                                    stop=(not last and mi == 3),
                                )
                        for si in range(8):
                            s = 8 * sw + si
                            if last:
                                nc.tensor.matmul(
                                    pts_v[si][:], ones[0:1, 0:P], bv_sb[:],
                                    start=False, stop=True)
                            if first:
                                nc.vector.tensor_copy(vacc[s][:], pts_v[si][:])
                            else:
                                nc.vector.tensor_tensor(
                                    vacc[s][:], vacc[s][:], pts_v[si][:],
                                    mybir.AluOpType.add)

                    # --- Q slabs: waves of (4 heads x 2 n-tiles of 512) ---
                    for nw in range(2):
                        pts_q = [[pp.tile([P, NT], F32, tag="pw",
                                          name=f"qps{g}_{nw}_{h}_{nn}")
                                  for nn in range(2)] for h in range(H_PER_CORE)]
                        for mi in range(4):
                            m = 4 * g + mi
                            wqt = wst.tile([P, 2 * NT], F32R, tag="wq",
                                           name=f"wq{g}_{nw}_{m}")
                            nc.sync.dma_start(
                                wqt[:], wq[ds(P * m, P), ds(2 * NT * nw, 2 * NT)])
                            for h in range(H_PER_CORE):
                                for nn in range(2):
                                    nc.tensor.matmul(
                                        pts_q[h][nn][:],
                                        xsl[mi][:, ds(P * h, P)],
                                        wqt[:, ds(NT * nn, NT)],
                                        start=(mi == 0),
                                        stop=(not last and mi == 3),
                                    )
                        for h in range(H_PER_CORE):
                            for nn in range(2):
                                n = 2 * nw + nn
                                if last:
                                    nc.tensor.matmul(
                                        pts_q[h][nn][:], ones[0:1, 0:P],
                                        bq_sb[:, ds(NT * n, NT)],
                                        start=False, stop=True)
                                dst = slabs[h][:, ds(NT * n, NT)]
                                if first:
                                    nc.vector.tensor_copy(dst, pts_q[h][nn][:])
                                else:
                                    nc.vector.tensor_tensor(
                                        dst, dst, pts_q[h][nn][:],
                                        mybir.AluOpType.add)

            # --- transpose slabs into tq[h][d, j, r] (q_h^T scrambled form) ---
            with ExitStack() as ph1b:
                tps = ph1b.enter_context(
                    tc.tile_pool(name="tps", bufs=4, space="PSUM"))
                for h in range(H_PER_CORE):
                    for j in range(M_CHUNKS):
                        tp = tps.tile([P, P], F32, tag="tp", name=f"tp{h}_{j}")
                        nc.tensor.transpose(
                            tp[:], slabs[h][:, ds(P * j, P)], ident[:])
                        nc.vector.tensor_copy(tq[h][:, j, :], tp[:])

        # ------------------------------------------------------------------
        # Phase 2: causal attention per (head, query-tile of 512).
        # ------------------------------------------------------------------
        oT_pool = top.enter_context(tc.tile_pool(name="oT", bufs=1))
        outT = [[None] * TT_TILES for _ in range(H_PER_CORE)]
        with ExitStack() as ph2:
            att = ph2.enter_context(tc.tile_pool(name="att", bufs=4))
            ps_s = ph2.enter_context(tc.tile_pool(name="ps_s", bufs=2, space="PSUM"))
            ps_u = ph2.enter_context(tc.tile_pool(name="ps_u", bufs=2, space="PSUM"))
            ps_d = ph2.enter_context(tc.tile_pool(name="ps_d", bufs=2, space="PSUM"))
            ps_b = ph2.enter_context(tc.tile_pool(name="ps_b", bufs=2, space="PSUM"))

            for h in range(H_PER_CORE):
                for tt in range(TT_TILES):
                    qap = tq[h][:, :, ds(32 * tt, 32)].rearrange("d j r -> d r j")
                    n_chunks = 4 * (tt + 1)
                    u_ps = ps_u.tile([P, NT], F32, tag="u", name=f"u{h}_{tt}")
                    d_ps = ps_d.tile([1, NT], F32, tag="d", name=f"d{h}_{tt}")
                    for c in range(n_chunks):
                        s_ps = ps_s.tile([P, NT], F32, tag="s",
                                         name=f"s{h}_{tt}_{c}")
                        nc.tensor.matmul(
                            s_ps[:], kacc[h][:, ds(P * c, P)], qap,
                            start=True, stop=True)
                        e_sb = att.tile([P, NT], F32R, tag="e",
                                        name=f"e{h}_{tt}_{c}")
                        nc.scalar.activation(
                            e_sb[:], s_ps[:],
                            mybir.ActivationFunctionType.Exp, scale=SCALE)
                        delta = c - 4 * tt
                        if delta >= 0:
                            # keep where t_loc >= s_loc + 128*delta, else 0
                            nc.gpsimd.affine_select(
                                out=e_sb[:], in_=e_sb[:],
                                compare_op=mybir.AluOpType.is_ge,
                                fill=0.0,
                                base=-128 * delta,
                                pattern=[[1, NT]],
                                channel_multiplier=-1,
                            )
                        nc.tensor.matmul(
                            d_ps[:], ones[:, 0:1], e_sb[:],
                            start=(c == 0), stop=(c == n_chunks - 1))
                        nc.tensor.matmul(
                            u_ps[:], vacc[c][:, ds(DH * h, DH)], e_sb[:],
                            start=(c == 0), stop=(c == n_chunks - 1))
                    den_sb = att.tile([1, NT], F32, tag="den", name=f"den{h}_{tt}")
                    nc.vector.tensor_copy(den_sb[:], d_ps[:])
                    rec_sb = att.tile([1, NT], F32R, tag="rec", name=f"rec{h}_{tt}")
                    with nc.allow_low_precision(
                            reason="f32r storage of fp32 reciprocal"):
                        nc.vector.reciprocal(rec_sb[:], den_sb[:])
                    b_ps = ps_b.tile([P, NT], F32, tag="b", name=f"b{h}_{tt}")
                    nc.tensor.matmul(b_ps[:], ones[0:1, 0:P], rec_sb[:],
                                     start=True, stop=True)
                    bc_sb = att.tile([P, NT], F32, tag="bc", name=f"bc{h}_{tt}")
                    nc.vector.tensor_copy(bc_sb[:], b_ps[:])
                    o_sb = oT_pool.tile([P, NT], F32R, name=f"oT{h}_{tt}")
                    nc.vector.tensor_tensor(
                        o_sb[:], u_ps[:], bc_sb[:], mybir.AluOpType.mult)
                    outT[h][tt] = o_sb

        # ------------------------------------------------------------------
        # Phase 3: partial output projection O[t, e] = sum_h out_h @ Wo_h.
        # ------------------------------------------------------------------
        with ExitStack() as ph3:
            wop = ph3.enter_context(tc.tile_pool(name="wop", bufs=1))
            ps_o = ph3.enter_context(tc.tile_pool(name="ps_o", bufs=6, space="PSUM"))
            ost = ph3.enter_context(tc.tile_pool(name="ost", bufs=4))
            wots = []
            for h in range(H_PER_CORE):
                wot = wop.tile([P, D], F32R, name=f"wo{h}")
                nc.sync.dma_start(wot[:], wo[ds(P * h, P), :])
                wots.append(wot)
            for tt in range(TT_TILES):
                for k in range(4):  # 128-row query block within the 512 tile
                    for e in range(TT_TILES):
                        o_ps = ps_o.tile([P, NT], F32, tag="o",
                                         name=f"o{tt}_{k}_{e}")
                        for h in range(H_PER_CORE):
                            nc.tensor.matmul(
                                o_ps[:],
                                outT[h][tt][:, ds(P * k, P)],
                                wots[h][:, ds(NT * e, NT)],
                                start=(h == 0), stop=(h == H_PER_CORE - 1))
                        o_sb = ost.tile([P, NT], F32, tag="os",
                                        name=f"os{tt}_{k}_{e}")
                        nc.vector.tensor_copy(o_sb[:], o_ps[:])
                        nc.sync.dma_start(
                            out[ds(NT * tt + P * k, P), ds(NT * e, NT)], o_sb[:])

    nc.finalize()
    return nc


def kernel(x, Wq, bq, Wk, bk, Wv, bv, Wo, bo):
    x = np.asarray(x, dtype=np.float32)
    Wq = np.ascontiguousarray(np.asarray(Wq, dtype=np.float32))
    Wk = np.asarray(Wk, dtype=np.float32)
    Wv = np.asarray(Wv, dtype=np.float32)
    Wo = np.asarray(Wo, dtype=np.float32)
    bq_ = np.ascontiguousarray(np.asarray(bq, dtype=np.float32).reshape(1, -1))
    bk_ = np.asarray(bk, dtype=np.float32).reshape(1, -1)
    bv_ = np.asarray(bv, dtype=np.float32).reshape(1, -1)
    bo_ = np.asarray(bo, dtype=np.float32)

    if "nc" not in _CACHE:
        _CACHE["nc"] = _build()
    nc = _CACHE["nc"]

    in_maps = []
    for c in range(8):
        b, g = c // 4, c % 4
        cols = slice(512 * g, 512 * (g + 1))
        xt = np.ascontiguousarray(x[b].T)
        in_maps.append({
            "x_t": xt,
            "x_slab": np.ascontiguousarray(xt[:, cols]),
            "wq": Wq,
            "wk": np.ascontiguousarray(Wk[:, cols]),
            "wv": np.ascontiguousarray(Wv[:, cols]),
            "wo": np.ascontiguousarray(Wo[cols, :]),
            "bq": bq_,
            "bk": np.ascontiguousarray(bk_[:, cols]),
            "bv": np.ascontiguousarray(bv_[:, cols]),
        })

    res = run_bass_kernel_spmd(nc, in_maps, core_ids=list(range(8)))
    _CACHE["last_results"] = res

    out = np.zeros((x.shape[0], T, D), dtype=np.float32)
    for b in range(x.shape[0]):
        acc_np = np.zeros((T, D), dtype=np.float32)
        for g in range(4):
            acc_np += res.results[4 * b + g]["out"]
        out[b] = acc_np + bo_[None, :]
    return out
